# revision 47
# baseline (speedup 1.0000x reference)
"""Causal self-attention (B=2, S=2048, D=1024, H=16) on 8 TRN2 NeuronCores.

Sharding: data-parallel over batch (2) x tensor-parallel over head groups
(4 groups of 4 heads).  Core c handles batch c//4, heads 4*(c%4)..4*(c%4)+3.
Each core computes its heads' QKV projection, causal attention, and a
partial output projection; the host sums the 4 head-group partials per
batch (the usual tensor-parallel all-reduce, done on host since outputs
are gathered anyway) and adds b_out.

On-chip layout (no transposes on device; host pre-transposes x):
  xT   [1152, 2048]  x[b]^T with a ones-row at 1024 (folds b_qkv in)
  Q^T  [256, S]  K^T [256, S]   feature-major (from lhsT=w, rhs=xT)
  V    [S, 260]  token-major, with a ones-column per head -> the E@V
                 matmul's row 64 yields the softmax denominator for free.
  scores are computed transposed: S^T[j,i] = K^T.T @ Q^T (head pairs share
  one wide PSUM tile and one ScalarE exp), causal masking only touches the
  single 128x128 triangle tile per diagonal block (columns left of the
  diagonal are never computed), then attn^T = (E^T).T-contracted against V
  via lhsT=V_aug.  Normalization (x 1/denom) happens after E@V
  (flash-style): recip = exp(-ln(denom)) on ScalarE (one combined Exp+Ln
  table load up front), broadcast across partitions by a K=1 matmul.
All matmuls run as float32r (TF32-class full-rate fp32 mode, N>=256).
"""

import os
import sys

import numpy as np

for _p in ("/root/.axon_site/_ro/trn_rl_repo", "/opt/trn_rl_repo"):
    if _p not in sys.path and os.path.isdir(_p):
        sys.path.append(_p)

import concourse.bacc as bacc
import concourse.bass as bass
import concourse.mybir as mybir
import concourse.tile as tile
from concourse.bass import ts
from concourse.bass_utils import run_bass_kernel_spmd

F32 = mybir.dt.float32
F32R = mybir.dt.float32r
EXP = mybir.ActivationFunctionType.Exp
LOG = mybir.ActivationFunctionType.Ln if hasattr(mybir.ActivationFunctionType, "Ln") else mybir.ActivationFunctionType.Log

B = 2
S = 2048
C = 1024
H = 16
DK = 64
NCORES = 8
HPC = 4          # heads per core
GROUPS = 4       # head groups (tensor-parallel)
FQ = HPC * DK    # 256 per-core q/k/v feature width
VW = HPC * 65    # V block width in wA incl. per-head ones column (260)
WAW = 2 * FQ + VW  # wA total width (772)
CK = C + 128     # contraction rows incl. bias row, padded to 128 mult
NKT = CK // 128  # 9 contraction tiles
NCI = S // 512   # 4 query chunks of 512
NTT = S // 128   # 16 token tiles


def _r(ap):
    return ap


def build_attention(nc, S=S, CK=CK, out_name="out"):
    """Emit the per-core attention program (SPMD; cores differ only in data)."""
    NKT = CK // 128
    NCI = S // 512
    NTT = S // 128

    xT = nc.dram_tensor("xT", [CK, S], F32R, kind="ExternalInput").ap()
    wA = nc.dram_tensor("wA", [CK, WAW], F32R, kind="ExternalInput").ap()
    wO = nc.dram_tensor("wO", [FQ, C], F32R, kind="ExternalInput").ap()
    tri = nc.dram_tensor("tri", [128, 256], F32R, kind="ExternalInput").ap()
    ones_d = nc.dram_tensor("ones_d", [128, 64], F32R, kind="ExternalInput").ap()
    out = nc.dram_tensor(out_name, [S, C], F32, kind="ExternalOutput").ap()

    with tile.TileContext(nc) as tc:
        from contextlib import ExitStack

        # One combined Exp+Ln+Copy table load up front; suppresses the
        # per-function auto-inserted loads (Exp<->Ln would thrash otherwise).
        try:
            from concourse.hw_specs import get_activation_tables
            _sets = list(get_activation_tables(nc.m.arch).keys())
            _sid = _sets.index("natural_log_exp_and_others")
            nc.scalar.add_instruction(mybir.InstLoadActFuncSet(
                name=nc.get_next_instruction_name(), ins=[], outs=[],
                act_func_set_id=_sid))
        except Exception:
            pass

        with ExitStack() as ctx:
            # ---- persistent tiles ----
            pers = ctx.enter_context(tc.tile_pool(name="pers", bufs=1))
            qk_sb = [pers.tile([128, S], F32R, name=f"qk{i}", tag=f"qk{i}") for i in range(4)]
            v_sb = [pers.tile([128, HPC * 65], F32R, name=f"v{t}", tag=f"v{t}") for t in range(NTT)]
            mask_sb = pers.tile([128, 256], F32R, name="mask", tag="mask")
            wo_sb = pers.tile([128, 2 * C], F32R, name="wo", tag="wo")
            ones_sb = pers.tile([65, 64], F32R, name="ones", tag="ones")


            # ---- phase 1: projections ----
            with ExitStack() as p1:
                xw = p1.enter_context(tc.tile_pool(name="xw", bufs=1))
                xt = [xw.tile([128, S], F32R, name=f"xt{k}", tag=f"xt{k}") for k in range(NKT)]
                wa = [xw.tile([128, WAW], F32R, name=f"wa{k}", tag=f"wa{k}") for k in range(NKT)]
                for k in range(NKT):
                    nc.sync.dma_start(wa[k][:, 0 : 2 * FQ],
                                      wA[128 * k : 128 * (k + 1), 0 : 2 * FQ])
                    nc.sync.dma_start(
                        xt[k][:, ts(0, 512)],
                        xT[128 * k : 128 * (k + 1), ts(0, 512)],
                    )
                for k in range(NKT):
                    nc.sync.dma_start(wa[k][:, 2 * FQ : WAW],
                                      wA[128 * k : 128 * (k + 1), 2 * FQ : WAW])
                nc.sync.dma_start(mask_sb[:, :], tri)
                wo4 = wO.rearrange("(a e d) n -> d e a n", a=2, e=2)
                nc.sync.dma_start(
                    wo_sb[64:128, :].rearrange("p (a n) -> p a n", a=2),
                    wo4[:, 0, :, :],
                )
                nc.sync.dma_start(
                    wo_sb[0:64, :].rearrange("p (a n) -> p a n", a=2),
                    wo4[:, 1, :, :],
                )
                nc.sync.dma_start(ones_sb[64:65, :], ones_d[0:1, 0:64])
                for ci in range(1, NCI):
                    for k in range(NKT):
                        nc.sync.dma_start(
                            xt[k][:, ts(ci, 512)],
                            xT[128 * k : 128 * (k + 1), ts(ci, 512)],
                        )

                ps_qk = p1.enter_context(tc.tile_pool(name="ps_qk", bufs=4, space="PSUM"))
                ps_v = p1.enter_context(tc.tile_pool(name="ps_v", bufs=2, space="PSUM"))

                # Q^T (ft 0,1) and K^T (ft 2,3); q-part of wA is pre-scaled 1/8
                for ci in range(NCI):
                    psf = [ps_qk.tile([128, 512], F32, tag="psqk",
                                      name=f"psqk{ci}_{f}") for f in range(4)]
                    for k in range(NKT):
                        for ft in range(4):
                            nc.tensor.matmul(
                                psf[ft][:, :],
                                _r(wa[k][:, ts(ft, 128)]),
                                _r(xt[k][:, ts(ci, 512)]),
                                start=(k == 0),
                                stop=(k == NKT - 1),
                            )
                    for ft in range(4):
                        nc.scalar.copy(qk_sb[ft][:, ts(ci, 512)], psf[ft][:, :])
                    # V token-major (ones columns come from wA's aug block)
                    for tt in range(4 * ci, 4 * ci + 4):
                        ps = ps_v.tile([128, VW], F32, tag="psv")
                        for k in range(NKT):
                            nc.tensor.matmul(
                                ps[:, :],
                                _r(xt[k][:, ts(tt, 128)]),
                                _r(wa[k][:, 2 * FQ : 2 * FQ + VW]),
                                start=(k == 0),
                                stop=(k == NKT - 1),
                            )
                        nc.vector.tensor_copy(v_sb[tt][:, :], ps[:, :])

            # ---- phase 2: attention + output projection ----
            with ExitStack() as p2:
                ps_s = p2.enter_context(tc.tile_pool(name="ps_s", bufs=2, space="PSUM"))
                ps_a = p2.enter_context(tc.tile_pool(name="ps_a", bufs=2, space="PSUM"))
                ps_o = p2.enter_context(tc.tile_pool(name="ps_o", bufs=2, space="PSUM"))
                ep = p2.enter_context(tc.tile_pool(name="ep", bufs=10))
                aup = p2.enter_context(tc.tile_pool(name="aup", bufs=8))
                anp = p2.enter_context(tc.tile_pool(name="anp", bufs=8))
                rtp = p2.enter_context(tc.tile_pool(name="rtp", bufs=8))
                op = p2.enter_context(tc.tile_pool(name="op", bufs=6))

                for ci in range(NCI):
                    att_p = [None] * (HPC // 2)
                    aus = []
                    njt = 4 * ci + 4
                    for hp in range(HPC // 2):
                        h0 = 2 * hp
                        kt_tile = qk_sb[2 + hp]
                        qt_tile = qk_sb[hp]
                        pa = [ps_a.tile([128, 512], F32, tag="psa", name=f"pa{ci}_{hp}_{e}")
                              for e in range(2)]
                        for jt in range(njt):
                            kd = jt - 4 * ci
                            lo = max(kd, 0) * 128  # first valid column
                            w = 512 - lo
                            pss = ps_s.tile([128, 1024], F32, tag="pss")
                            et = ep.tile([128, 1024], F32R, tag="et")
                            for e in range(2):
                                nc.tensor.matmul(
                                    pss[:, 512 * e + lo : 512 * (e + 1)],
                                    _r(kt_tile[64 * e : 64 * e + 64, ts(jt, 128)]),
                                    _r(qt_tile[64 * e : 64 * e + 64,
                                               512 * ci + lo : 512 * (ci + 1)]),
                                    start=True,
                                    stop=True,
                                )
                            # one exp over both heads' valid columns (3D AP)
                            nc.scalar.activation(
                                et.rearrange("p (e c) -> p e c", e=2)[:, :, lo:512],
                                pss.rearrange("p (e c) -> p e c", e=2)[:, :, lo:512],
                                EXP,
                            )
                            if kd >= 0:
                                nc.vector.tensor_mul(
                                    et.rearrange("p (e c) -> p e c", e=2)[:, :, lo : lo + 128],
                                    et.rearrange("p (e c) -> p e c", e=2)[:, :, lo : lo + 128],
                                    mask_sb.rearrange("p (e c) -> p e c", e=2),
                                )
                            for e in range(2):
                                nc.tensor.matmul(
                                    pa[e][0:65, lo:512],
                                    _r(v_sb[jt][:, 65 * (h0 + e) : 65 * (h0 + e) + 65]),
                                    _r(et[:, 512 * e + lo : 512 * (e + 1)]),
                                    start=(jt == 0),
                                    stop=(jt == njt - 1),
                                )
                        # free the accumulators now; normalize after both pairs
                        # so the Ln/Exp ops don't contend with score-exps on ACT
                        for e in range(2):
                            au = aup.tile([65, 512], F32, tag="au",
                                          name=f"au{ci}_{hp}_{e}")
                            nc.vector.tensor_copy(au[:, :], pa[e][0:65, :])
                            aus.append(au)

                    for hp in range(HPC // 2):
                        an_pair = anp.tile([128, 512], F32R, tag="an",
                                           name=f"anp{ci}_{hp}")
                        for e in range(2):
                            au = aus[2 * hp + e]
                            rec = rtp.tile([65, 512], F32, tag="rec",
                                           name=f"rc{ci}_{hp}_{e}")
                            nc.scalar.activation(rec[64:65, :], au[64:65, :], LOG)
                            recr = rtp.tile([65, 512], F32R, tag="recr",
                                            name=f"rr{ci}_{hp}_{e}")
                            nc.scalar.activation(recr[64:65, :], rec[64:65, :], EXP,
                                                 scale=-1.0)
                            pbx = ps_o.tile([64, 512], F32, tag="pso",
                                            name=f"pb{ci}_{hp}_{e}")
                            nc.tensor.matmul(
                                pbx[:, :],
                                _r(ones_sb[64:65, :]),
                                _r(recr[64:65, :]),
                                start=True,
                                stop=True,
                            )
                            if e == 1:
                                nc.vector.tensor_mul(an_pair[0:64, :], au[0:64, :],
                                                     pbx[:, :])
                            else:
                                # DVE can't cross lanes; normalize in place then
                                # DMA-shift this head to partitions 64-127
                                antmp = anp.tile([64, 512], F32R, tag="antmp",
                                                 name=f"at{ci}_{hp}")
                                nc.vector.tensor_mul(antmp[:, :], au[0:64, :],
                                                     pbx[:, :])
                                nc.sync.dma_start(an_pair[64:128, :], antmp[:, :])
                        att_p[hp] = an_pair

                    for it in range(4):
                        ot = op.tile([128, 1024], F32, tag="ot")
                        for nch in range(2):
                            po = ps_o.tile([128, 512], F32, tag="pso",
                                           name=f"po{ci}_{it}_{nch}")
                            for hp in range(HPC // 2):
                                nc.tensor.matmul(
                                    po[:, :],
                                    _r(att_p[hp][:, ts(it, 128)]),
                                    _r(wo_sb[:, C * hp + 512 * nch : C * hp + 512 * (nch + 1)]),
                                    start=(hp == 0),
                                    stop=(hp == HPC // 2 - 1),
                                )
                            nc.vector.tensor_copy(ot[:, ts(nch, 512)], po[:, :])
                            nc.sync.dma_start(
                                out[512 * ci + 128 * it : 512 * ci + 128 * (it + 1),
                                    ts(nch, 512)],
                                ot[:, ts(nch, 512)],
                            )
    return nc


_CACHE = {}


def _get_compiled():
    if "nc" not in _CACHE:
        nc = bacc.Bacc("TRN2", target_bir_lowering=False, debug=False,
                       num_devices=NCORES)
        build_attention(nc)
        nc.compile()
        _CACHE["nc"] = nc
    return _CACHE["nc"]


def _mask4():
    jl = np.arange(128)[:, None]
    il = np.arange(128)[None, :]
    t = (jl <= il).astype(np.float32)
    return np.concatenate([t, t], axis=1)


def _prep_core(x, w_qkv, b_qkv, w_out, b, g, mask4):
    xT = np.zeros((CK, S), dtype=np.float32)
    xT[:C] = x[b].T
    xT[C] = 1.0
    qc = slice(FQ * g, FQ * (g + 1))
    kc = slice(C + FQ * g, C + FQ * (g + 1))
    vc = slice(2 * C + FQ * g, 2 * C + FQ * (g + 1))
    wA = np.zeros((CK, WAW), dtype=np.float32)
    wA[:C, 0:FQ] = w_qkv[:, qc] * 0.125
    wA[:C, FQ : 2 * FQ] = w_qkv[:, kc]
    wA[C, 0:FQ] = b_qkv[qc] * 0.125
    wA[C, FQ : 2 * FQ] = b_qkv[kc]
    wv = wA[:, 2 * FQ :].reshape(CK, HPC, 65)
    wv[:C, :, 0:64] = w_qkv[:, vc].reshape(C, HPC, 64)
    wv[C, :, 0:64] = b_qkv[vc].reshape(HPC, 64)
    wv[C, :, 64] = 1.0
    # row order (h_local*64+d) = (hp*128 + e*64 + d) already matches the
    # paired (a=hp, p=(e,d)) DMA layout -- no reorder needed
    wO = np.ascontiguousarray(w_out[FQ * g : FQ * (g + 1), :], dtype=np.float32)
    return {"xT": xT, "wA": wA, "wO": wO, "tri": mask4,
            "ones_d": np.ones((128, 64), dtype=np.float32)}


def kernel(x, mask, w_qkv, b_qkv, w_out, b_out):
    x = np.asarray(x, dtype=np.float32)
    w_qkv = np.asarray(w_qkv, dtype=np.float32)
    b_qkv = np.asarray(b_qkv, dtype=np.float32)
    w_out = np.asarray(w_out, dtype=np.float32)
    b_out = np.asarray(b_out, dtype=np.float32)

    # the axon NTFF trace path is absent in this container; make sure an
    # inherited BASS_TRACE can't send run_bass_kernel_spmd down it
    os.environ["BASS_NEVER_TRACE"] = "1"
    nc = _get_compiled()
    m4 = _mask4()
    in_maps = []
    for c in range(NCORES):
        b, g = divmod(c, GROUPS)
        in_maps.append(_prep_core(x, w_qkv, b_qkv, w_out, b, g, m4))

    res = run_bass_kernel_spmd(nc, in_maps, core_ids=list(range(NCORES)))

    outf = np.zeros((B, S, C), dtype=np.float32)
    for c in range(NCORES):
        b, g = divmod(c, GROUPS)
        outf[b] += res.results[c]["out"]
    outf += b_out[None, None, :]
    return outf
